# revision 4
# baseline (speedup 1.0000x reference)
"""Multi-head attention kernel for 8 TRN2 NeuronCores (Bass/Tile).

Problem: x[2,2048,1024], 16 heads x 64 dims, torch-style Linear weights.
Sharding: data parallel over batch (2) x tensor parallel over heads (16/4):
core c handles batch c//4, heads 4*(c%4) .. 4*(c%4)+3. Each core computes
its heads' attention output projected through its slice of wo, producing a
partial [2048, 1024] fp32 output; the host sums the 4 partials per batch
(the "all-reduce after wo").

Device dataflow per core (all matmul operands bf16, fp32 accumulation):
  QT/KT = (wq/wk slice)^T-projections in [d, t] layout (d on partitions)
  V     = projection in natural [s, d] layout, with a ones column appended
          per head so the P@V matmul also yields the softmax denominator
  S^T   = K^T.T @ Q^T per head ([s, t] layout, s on partitions)
  P^T   = exp(S^T / 8) via ScalarE (no max subtraction: logits are O(8))
  O^T   = V.T @ P^T accumulated over s in PSUM (row 64 = denominator)
  y     = (O^T / denom)^T @ wo-slice^T, partial over this core's heads
"""

import sys

sys.path.insert(0, "/opt/trn_rl_repo")

from contextlib import ExitStack

import ml_dtypes
import numpy as np

import concourse.bass as bass
import concourse.tile as tile
from concourse import bacc, mybir
from concourse import bass_utils
from concourse.bass_interp import get_hw_module

BF16 = mybir.dt.bfloat16
F32 = mybir.dt.float32

N_EMBD = 1024
N_HEAD = 16
HEAD_DIM = 64

N_CORES = 8
HEADS_PER_CORE = 4  # 4 heads x 64 dims = 256 dims per core
DH = HEADS_PER_CORE * HEAD_DIM  # 256


def build_program(T=2048, C=N_EMBD, enable_asserts=False):
    """Build the single-core Bass program (SPMD across 8 cores)."""
    nc = bacc.Bacc(
        "TRN2", target_bir_lowering=False, debug=False, enable_asserts=enable_asserts
    )

    xT = nc.dram_tensor("xT", [C, T], BF16, kind="ExternalInput").ap()
    wqT = nc.dram_tensor("wqT", [C, DH], BF16, kind="ExternalInput").ap()
    wkT = nc.dram_tensor("wkT", [C, DH], BF16, kind="ExternalInput").ap()
    wvT = nc.dram_tensor("wvT", [C, DH], BF16, kind="ExternalInput").ap()
    woT = nc.dram_tensor("woT", [DH, C], BF16, kind="ExternalInput").ap()
    y = nc.dram_tensor("y", [T, C], F32, kind="ExternalOutput").ap()

    n_ct = C // 128  # contraction tiles over embedding dim
    n_st = T // 128  # s tiles (key/value positions)
    n_tb = T // 512  # t blocks (query positions)
    n_cb = C // 512  # output column blocks
    n_tt = T // 128  # t tiles for the final projection

    with tile.TileContext(nc) as tc, ExitStack() as ctx:
        statics = ctx.enter_context(tc.tile_pool(name="statics", bufs=1))
        copies = ctx.enter_context(tc.tile_pool(name="copies", bufs=4))
        pt_pool = ctx.enter_context(tc.tile_pool(name="pt", bufs=8))
        small = ctx.enter_context(tc.tile_pool(name="small", bufs=4))
        out_stage = ctx.enter_context(tc.tile_pool(name="out_stage", bufs=4))

        psum_mm = ctx.enter_context(tc.tile_pool(name="psum_mm", bufs=4, space="PSUM"))
        psum_o = ctx.enter_context(tc.tile_pool(name="psum_o", bufs=2, space="PSUM"))
        psum_r = ctx.enter_context(tc.tile_pool(name="psum_r", bufs=1, space="PSUM"))

        # ---- static SBUF tensors ----
        xT_sb = statics.tile([128, n_ct, T], BF16)
        wq_sb = statics.tile([128, n_ct, DH], BF16)
        wk_sb = statics.tile([128, n_ct, DH], BF16)
        wv_sb = statics.tile([128, n_ct, DH], BF16)
        wo_sb = statics.tile([128, 2, C], BF16)
        qT_sb = statics.tile([128, 2, T], BF16)
        kT_sb = statics.tile([128, 2, T], BF16)
        v_sb = statics.tile([128, n_st, HEADS_PER_CORE, HEAD_DIM + 1], BF16)
        oT_sb = statics.tile([128, 2, T], BF16)
        ones_sb = statics.tile([1, 64], BF16)

        nc.sync.dma_start(out=xT_sb, in_=xT.rearrange("(a p) t -> p a t", p=128))
        nc.sync.dma_start(out=wq_sb, in_=wqT.rearrange("(a p) d -> p a d", p=128))
        nc.sync.dma_start(out=wk_sb, in_=wkT.rearrange("(a p) d -> p a d", p=128))
        nc.sync.dma_start(out=wv_sb, in_=wvT.rearrange("(a p) d -> p a d", p=128))
        nc.sync.dma_start(out=wo_sb, in_=woT.rearrange("(a p) c -> p a c", p=128))
        nc.vector.memset(ones_sb, 1.0)
        nc.vector.memset(v_sb[:, :, :, HEAD_DIM : HEAD_DIM + 1], 1.0)

        # ---- QT / KT projections: [d, t] layout, d on partitions ----
        for w_sb, dst in ((wq_sb, qT_sb), (wk_sb, kT_sb)):
            for chunk in range(2):
                for tb in range(n_tb):
                    ps = psum_mm.tile([128, 512], F32, tag="mm")
                    for ct in range(n_ct):
                        nc.tensor.matmul(
                            ps,
                            lhsT=w_sb[:, ct, chunk * 128 : (chunk + 1) * 128],
                            rhs=xT_sb[:, ct, tb * 512 : (tb + 1) * 512],
                            start=(ct == 0),
                            stop=(ct == n_ct - 1),
                        )
                    nc.vector.tensor_copy(
                        dst[:, chunk, tb * 512 : (tb + 1) * 512], ps
                    )

        # ---- V projection: natural [s, d] layout plus ones column ----
        for st in range(n_st):
            ps = psum_mm.tile([128, 512], F32, tag="mm")
            for ct in range(n_ct):
                nc.tensor.matmul(
                    ps[:, 0:DH],
                    lhsT=xT_sb[:, ct, st * 128 : (st + 1) * 128],
                    rhs=wv_sb[:, ct, :],
                    start=(ct == 0),
                    stop=(ct == n_ct - 1),
                )
            nc.vector.tensor_copy(
                v_sb[:, st, :, 0:HEAD_DIM],
                ps[:, 0:DH].rearrange("p (h d) -> p h d", h=HEADS_PER_CORE),
            )

        # ---- attention per head ----
        for h in range(HEADS_PER_CORE):
            chunk, dlo = h // 2, (h % 2) * 64
            for tb in range(n_tb):
                o_ps = psum_o.tile([HEAD_DIM + 1, 512], F32)
                for st in range(n_st):
                    s_ps = psum_mm.tile([128, 512], F32, tag="mm")
                    nc.tensor.matmul(
                        s_ps,
                        lhsT=kT_sb[dlo : dlo + 64, chunk, st * 128 : (st + 1) * 128],
                        rhs=qT_sb[dlo : dlo + 64, chunk, tb * 512 : (tb + 1) * 512],
                        start=True,
                        stop=True,
                    )
                    pt = pt_pool.tile([128, 512], BF16)
                    nc.scalar.activation(
                        pt, s_ps, mybir.ActivationFunctionType.Exp,
                        scale=float(HEAD_DIM**-0.5),
                    )
                    nc.tensor.matmul(
                        o_ps,
                        lhsT=v_sb[:, st, h, :],
                        rhs=pt,
                        start=(st == 0),
                        stop=(st == n_st - 1),
                    )
                # normalize: row 64 of o_ps is the softmax denominator
                recip = small.tile([1, 512], BF16)
                with nc.allow_low_precision(reason="softmax denom reciprocal in bf16"):
                    nc.vector.reciprocal(recip, o_ps[64:65, :])
                rep = psum_r.tile([64, 512], F32)
                nc.tensor.matmul(rep, lhsT=ones_sb, rhs=recip, start=True, stop=True)
                rep_sb = small.tile([64, 512], BF16, tag="rep_sb")
                nc.vector.tensor_copy(rep_sb, rep)
                nc.vector.tensor_mul(
                    oT_sb[dlo : dlo + 64, chunk, tb * 512 : (tb + 1) * 512],
                    o_ps[0:64, :],
                    rep_sb,
                )

        # ---- final projection: y[t, c] partial ----
        y_tiled = y.rearrange("(tt p) c -> p tt c", p=128)
        for tt in range(n_tt):
            for cb in range(n_cb):
                ps = psum_mm.tile([128, 512], F32, tag="mm")
                for chunk in range(2):
                    nc.tensor.matmul(
                        ps,
                        lhsT=oT_sb[:, chunk, tt * 128 : (tt + 1) * 128],
                        rhs=wo_sb[:, chunk, cb * 512 : (cb + 1) * 512],
                        start=(chunk == 0),
                        stop=(chunk == 1),
                    )
                st_out = out_stage.tile([128, 512], F32)
                nc.vector.tensor_copy(st_out, ps)
                nc.sync.dma_start(
                    out=y_tiled[:, tt, cb * 512 : (cb + 1) * 512], in_=st_out
                )

    nc.compile()
    return nc


def make_core_inputs(x, wq, wk, wv, wo):
    """Shard + pre-layout the full inputs into 8 per-core input maps."""
    bf = ml_dtypes.bfloat16
    in_maps = []
    for core in range(N_CORES):
        b = core // 4
        g = core % 4
        lo, hi = g * DH, (g + 1) * DH
        in_maps.append(
            {
                "xT": np.ascontiguousarray(x[b].T).astype(bf),
                "wqT": np.ascontiguousarray(wq[lo:hi, :].T).astype(bf),
                "wkT": np.ascontiguousarray(wk[lo:hi, :].T).astype(bf),
                "wvT": np.ascontiguousarray(wv[lo:hi, :].T).astype(bf),
                "woT": np.ascontiguousarray(wo[:, lo:hi].T).astype(bf),
            }
        )
    return in_maps


_PROGRAM_CACHE = {}


def _get_program():
    if "nc" not in _PROGRAM_CACHE:
        nc = build_program()
        nc.m = get_hw_module(nc.m)
        _PROGRAM_CACHE["nc"] = nc
    return _PROGRAM_CACHE["nc"]


def run_sharded(in_maps, trace=False):
    nc = _get_program()
    return bass_utils.run_bass_kernel_spmd(
        nc, in_maps, core_ids=list(range(N_CORES)), trace=trace
    )


def kernel(x, wq, wk, wv, wo):
    x = np.asarray(x, dtype=np.float32)
    wq = np.asarray(wq, dtype=np.float32)
    wk = np.asarray(wk, dtype=np.float32)
    wv = np.asarray(wv, dtype=np.float32)
    wo = np.asarray(wo, dtype=np.float32)

    in_maps = make_core_inputs(x, wq, wk, wv, wo)
    res = run_sharded(in_maps)

    B, T, C = x.shape
    out = np.zeros((B, T, C), dtype=np.float32)
    for core in range(N_CORES):
        out[core // 4] += res.results[core]["y"]
    return out


if __name__ == "__main__":
    # quick self-run with random data
    rng = np.random.default_rng(0)
    x = rng.standard_normal((2, 2048, 1024), dtype=np.float32)
    s = 1.0 / np.sqrt(N_EMBD)
    ws = [rng.standard_normal((1024, 1024), dtype=np.float32) * s for _ in range(4)]
    out = kernel(x, *ws)
    print("out", out.shape, out.dtype, float(np.abs(out).max()))


# revision 10
# speedup vs baseline: 1.6254x; 1.6254x over previous
"""Multi-head attention kernel for 8 TRN2 NeuronCores (Bass/Tile).

Problem: x[2,2048,1024], 16 heads x 64 dims, torch-style Linear weights.
Sharding: data parallel over batch (2) x tensor parallel over heads (16/4):
core c handles batch c//4, heads 4*(c%4) .. 4*(c%4)+3. Each core computes
its heads' attention output projected through its slice of wo, producing a
partial [2048, 1024] fp32 output; the host sums the 4 partials per batch
(the "all-reduce after wo").

Device dataflow per core (matmul operands bf16, fp32 accumulation):
  QT/KT = weight-slice projections in [d, t] layout (d on partitions)
  V     = projection in natural [s, d] layout, with a ones column appended
          per head so the P@V matmul also yields the softmax denominator
  S^T   = K^T.T @ Q^T per head ([s, t] layout, s on partitions)
  P^T   = exp(S^T / 8) via ScalarE (no max subtraction: logits are O(8))
  O^T   = V.T @ P^T accumulated over s in PSUM (row 64 = denominator)
  y     = (O^T / denom)^T @ wo-slice^T, partial over this core's heads

The attention stage is ScalarE(exp)-bound (~16.8M exp/core), so the s-loop
is software-pipelined in chunks of 2 s-tiles: the PE issues chunk c+1's
score matmuls before chunk c's PV matmuls so the in-order PE queue never
stalls on the exp dependency, and each exp is one [128,1024] PSUM-source
ACTIVATE (amortizes the per-instruction bubble). Final projection and the
softmax normalization for query-block tb are deferred into tb+1's pipeline.
"""

import sys

sys.path.insert(0, "/opt/trn_rl_repo")

from contextlib import ExitStack

import ml_dtypes
import numpy as np

import concourse.bass as bass
import concourse.tile as tile
from concourse import bacc, mybir
from concourse import bass_utils
from concourse.bass_interp import get_hw_module

BF16 = mybir.dt.bfloat16
F32 = mybir.dt.float32
EXP = mybir.ActivationFunctionType.Exp

N_EMBD = 1024
N_HEAD = 16
HEAD_DIM = 64

N_CORES = 8
HEADS_PER_CORE = 4
DH = HEADS_PER_CORE * HEAD_DIM  # 256
CH = 2  # s-tiles per exp chunk


def build_program(T=2048, C=N_EMBD, enable_asserts=False):
    nc = bacc.Bacc(
        "TRN2", target_bir_lowering=False, debug=False, enable_asserts=enable_asserts
    )

    xT = nc.dram_tensor("xT", [C, T], BF16, kind="ExternalInput").ap()
    wqT = nc.dram_tensor("wqT", [C, DH], BF16, kind="ExternalInput").ap()
    wkT = nc.dram_tensor("wkT", [C, DH], BF16, kind="ExternalInput").ap()
    wvT = nc.dram_tensor("wvT", [C, DH], BF16, kind="ExternalInput").ap()
    woT = nc.dram_tensor("woT", [DH, C], BF16, kind="ExternalInput").ap()
    y = nc.dram_tensor("y", [T, C], F32, kind="ExternalOutput").ap()

    n_ct = C // 128   # contraction tiles over embedding dim
    n_st = T // 128   # s tiles (key/value positions)
    n_tb = T // 512   # query blocks
    n_cb = C // 512   # output column blocks
    n_ck = n_st // CH  # exp chunks per (h, tb)

    scale = float(HEAD_DIM**-0.5)

    with tile.TileContext(nc) as tc, ExitStack() as ctx:
        statics = ctx.enter_context(tc.tile_pool(name="statics", bufs=1))
        pt_pool = ctx.enter_context(tc.tile_pool(name="pt", bufs=4))
        onorm_pool = ctx.enter_context(tc.tile_pool(name="onorm", bufs=6))
        small = ctx.enter_context(tc.tile_pool(name="small", bufs=6))
        out_stage = ctx.enter_context(tc.tile_pool(name="out_stage", bufs=4))

        psum_s = ctx.enter_context(tc.tile_pool(name="psum_s", bufs=2, space="PSUM"))
        psum_o = ctx.enter_context(tc.tile_pool(name="psum_o", bufs=2, space="PSUM"))
        psum_f = ctx.enter_context(tc.tile_pool(name="psum_f", bufs=2, space="PSUM"))

        # ---- static SBUF tensors ----
        xT_sb = statics.tile([128, n_ct, T], BF16)
        wq_sb = statics.tile([128, n_ct, DH], BF16)
        wk_sb = statics.tile([128, n_ct, DH], BF16)
        wv_sb = statics.tile([128, n_ct, DH], BF16)
        wo_sb = statics.tile([128, 2, C], BF16)
        qT_sb = statics.tile([128, 2, T], BF16)
        kT_sb = statics.tile([128, 2, T], BF16)
        v_sb = statics.tile([128, n_st, HEADS_PER_CORE, HEAD_DIM + 1], BF16)
        oT_sb = statics.tile([128, 2, T], BF16)
        ones_sb = statics.tile([1, 64], BF16)

        nc.sync.dma_start(out=xT_sb, in_=xT.rearrange("(a p) t -> p a t", p=128))
        nc.sync.dma_start(out=wq_sb, in_=wqT.rearrange("(a p) d -> p a d", p=128))
        nc.sync.dma_start(out=wk_sb, in_=wkT.rearrange("(a p) d -> p a d", p=128))
        nc.sync.dma_start(out=wv_sb, in_=wvT.rearrange("(a p) d -> p a d", p=128))
        nc.sync.dma_start(out=wo_sb, in_=woT.rearrange("(a p) c -> p a c", p=128))
        nc.vector.memset(ones_sb, 1.0)
        nc.vector.memset(v_sb[:, :, :, HEAD_DIM : HEAD_DIM + 1], 1.0)

        # ---- projections (K, V first; attention waits on all of them) ----
        for w_sb, dst in ((wk_sb, kT_sb), (wq_sb, qT_sb)):
            for chunk in range(2):
                for tb in range(n_tb):
                    ps = psum_f.tile([128, 512], F32, tag="f")
                    for ct in range(n_ct):
                        nc.tensor.matmul(
                            ps,
                            lhsT=w_sb[:, ct, chunk * 128 : (chunk + 1) * 128],
                            rhs=xT_sb[:, ct, tb * 512 : (tb + 1) * 512],
                            start=(ct == 0),
                            stop=(ct == n_ct - 1),
                        )
                    nc.vector.tensor_copy(dst[:, chunk, tb * 512 : (tb + 1) * 512], ps)

        for st in range(n_st):
            ps = psum_f.tile([128, 512], F32, tag="f")
            for ct in range(n_ct):
                nc.tensor.matmul(
                    ps[:, 0:DH],
                    lhsT=xT_sb[:, ct, st * 128 : (st + 1) * 128],
                    rhs=wv_sb[:, ct, :],
                    start=(ct == 0),
                    stop=(ct == n_ct - 1),
                )
            nc.vector.tensor_copy(
                v_sb[:, st, :, 0:HEAD_DIM],
                ps[:, 0:DH].rearrange("p (h d) -> p h d", h=HEADS_PER_CORE),
            )

        # ---- attention: software-pipelined chunk loop ----
        # chunk records: (h, tb, c, ps, pt, o_ps); deferred: list of
        # (slot_index, emit_fn) executed at the start of pipeline slot i.
        heads = list(range(HEADS_PER_CORE))
        chunk_list = []
        for tb in range(n_tb):
            for h in heads:
                for c in range(n_ck):
                    chunk_list.append((tb, h, c))
        n_total = len(chunk_list)

        deferred = {}

        def defer(slot, fn):
            deferred.setdefault(slot, []).append(fn)

        o_ps_map = {}
        denom_map = {}

        def emit_S_ACT(i):
            tb, h, c = chunk_list[i]
            chunk_hd, dlo = h // 2, (h % 2) * 64
            if c == 0:
                o_ps = psum_o.tile([HEAD_DIM + 1, 512], F32, tag="o")
                o_ps_map[(tb, h)] = o_ps
            ps = psum_s.tile([128, CH * 512], F32, tag="s")
            for j in range(CH):
                st = c * CH + j
                nc.tensor.matmul(
                    ps[:, j * 512 : (j + 1) * 512],
                    lhsT=kT_sb[dlo : dlo + 64, chunk_hd, st * 128 : (st + 1) * 128],
                    rhs=qT_sb[dlo : dlo + 64, chunk_hd, tb * 512 : (tb + 1) * 512],
                    start=True,
                    stop=True,
                )
            pt = pt_pool.tile([128, CH, 512], BF16)
            nc.scalar.activation(
                pt.rearrange("p a b -> p (a b)"), ps, EXP, scale=scale
            )
            return ps, pt

        def emit_O(i, pt):
            tb, h, c = chunk_list[i]
            o_ps = o_ps_map[(tb, h)]
            for j in range(CH):
                st = c * CH + j
                nc.tensor.matmul(
                    o_ps,
                    lhsT=v_sb[:, st, h, :],
                    rhs=pt[:, j, :],
                    start=(c == 0 and j == 0),
                    stop=(c == n_ck - 1 and j == CH - 1),
                )
            if c == n_ck - 1:
                # head (tb, h) fully accumulated: stash unnormalized O
                o_unnorm = onorm_pool.tile([64, 512], BF16, tag="ou")
                nc.vector.tensor_copy(o_unnorm, o_ps[0:64, :])
                o_unnorm_map[(tb, h)] = o_unnorm

        o_unnorm_map = {}

        def emit_norm(tb, h):
            chunk_hd, dlo = h // 2, (h % 2) * 64
            o_ps = o_ps_map[(tb, h)]
            denom_f = small.tile([1, 512], F32, tag="denom_f")
            nc.vector.tensor_copy(denom_f, o_ps[64:65, :])
            recip_f = small.tile([1, 512], F32, tag="recip_f")
            nc.vector.reciprocal_approx_fast(recip_f, denom_f)
            recip = small.tile([1, 512], BF16, tag="recip")
            nc.vector.tensor_copy(recip, recip_f)
            rep = psum_f.tile([128, 512], F32, tag="f")
            nc.tensor.matmul(
                rep[0:64, :], lhsT=ones_sb, rhs=recip, start=True, stop=True
            )
            rep_sb = small.tile([64, 512], BF16, tag="rep")
            nc.vector.tensor_copy(rep_sb, rep[0:64, :])
            nc.vector.tensor_mul(
                oT_sb[dlo : dlo + 64, chunk_hd, tb * 512 : (tb + 1) * 512],
                o_unnorm_map[(tb, h)],
                rep_sb,
            )

        def emit_final(tb):
            # y partial for query block tb: 4 t-tiles of 128
            y_tiled = y.rearrange("(tt p) c -> p tt c", p=128)
            for tt in range(tb * 4, tb * 4 + 4):
                for cb in range(n_cb):
                    ps = psum_f.tile([128, 512], F32, tag="f")
                    for chunk in range(2):
                        nc.tensor.matmul(
                            ps,
                            lhsT=oT_sb[:, chunk, tt * 128 : (tt + 1) * 128],
                            rhs=wo_sb[:, chunk, cb * 512 : (cb + 1) * 512],
                            start=(chunk == 0),
                            stop=(chunk == 1),
                        )
                    st_out = out_stage.tile([128, 512], F32)
                    nc.vector.tensor_copy(st_out, ps)
                    nc.sync.dma_start(
                        out=y_tiled[:, tt, cb * 512 : (cb + 1) * 512], in_=st_out
                    )

        # schedule: at slot i emit S+ACT(i) then O(i-1); normalization for tb
        # lands ~3 chunks into tb+1, final projection ~6 chunks in.
        per_tb = HEADS_PER_CORE * n_ck
        for tb in range(n_tb):
            for h in heads:
                head_end = (tb * HEADS_PER_CORE + h + 1) * n_ck
                defer(min(head_end + 2, n_total), lambda tb=tb, h=h: emit_norm(tb, h))
            end_slot = (tb + 1) * per_tb
            defer(min(end_slot + 5, n_total), lambda tb=tb: emit_final(tb))

        pending = None
        for i in range(n_total + 1):
            if i < n_total:
                ps, pt = emit_S_ACT(i)
            if pending is not None:
                emit_O(pending[0], pending[1])
            for fn in deferred.get(i, ()):
                fn()
            pending = (i, pt) if i < n_total else None

    nc.compile()
    return nc


def make_core_inputs(x, wq, wk, wv, wo):
    """Shard + pre-layout the full inputs into 8 per-core input maps."""
    bf = ml_dtypes.bfloat16
    in_maps = []
    for core in range(N_CORES):
        b = core // 4
        g = core % 4
        lo, hi = g * DH, (g + 1) * DH
        in_maps.append(
            {
                "xT": np.ascontiguousarray(x[b].T).astype(bf),
                "wqT": np.ascontiguousarray(wq[lo:hi, :].T).astype(bf),
                "wkT": np.ascontiguousarray(wk[lo:hi, :].T).astype(bf),
                "wvT": np.ascontiguousarray(wv[lo:hi, :].T).astype(bf),
                "woT": np.ascontiguousarray(wo[:, lo:hi].T).astype(bf),
            }
        )
    return in_maps


_PROGRAM_CACHE = {}


def _get_program():
    if "nc" not in _PROGRAM_CACHE:
        nc = build_program()
        nc.m = get_hw_module(nc.m)
        _PROGRAM_CACHE["nc"] = nc
    return _PROGRAM_CACHE["nc"]


def run_sharded(in_maps, trace=False):
    nc = _get_program()
    return bass_utils.run_bass_kernel_spmd(
        nc, in_maps, core_ids=list(range(N_CORES)), trace=trace
    )


def kernel(x, wq, wk, wv, wo):
    x = np.asarray(x, dtype=np.float32)
    wq = np.asarray(wq, dtype=np.float32)
    wk = np.asarray(wk, dtype=np.float32)
    wv = np.asarray(wv, dtype=np.float32)
    wo = np.asarray(wo, dtype=np.float32)

    in_maps = make_core_inputs(x, wq, wk, wv, wo)
    res = run_sharded(in_maps)

    B, T, C = x.shape
    out = np.zeros((B, T, C), dtype=np.float32)
    for core in range(N_CORES):
        out[core // 4] += res.results[core]["y"]
    return out


if __name__ == "__main__":
    rng = np.random.default_rng(0)
    x = rng.standard_normal((2, 2048, 1024), dtype=np.float32)
    s = 1.0 / np.sqrt(N_EMBD)
    ws = [rng.standard_normal((1024, 1024), dtype=np.float32) * s for _ in range(4)]
    out = kernel(x, *ws)
    print("out", out.shape, out.dtype, float(np.abs(out).max()))


# revision 12
# speedup vs baseline: 1.7068x; 1.0501x over previous
"""Multi-head attention kernel for 8 TRN2 NeuronCores (Bass/Tile).

Problem: x[2,2048,1024], 16 heads x 64 dims, torch-style Linear weights.
Sharding: data parallel over batch (2) x tensor parallel over heads (16/4):
core c handles batch c//4, heads 4*(c%4) .. 4*(c%4)+3. Each core computes
its heads' attention output projected through its slice of wo, producing a
partial [2048, 1024] fp32 output; the host sums the 4 partials per batch
(the "all-reduce after wo").

Device dataflow per core (matmul operands bf16, fp32 accumulation):
  QT/KT = weight-slice projections in [d, t] layout (d on partitions)
  V     = projection in natural [s, d] layout, with a ones column appended
          per head so the P@V matmul also yields the softmax denominator
  S^T   = K^T.T @ Q^T per head ([s, t] layout, s on partitions)
  P^T   = exp(S^T / 8) via ScalarE (no max subtraction: logits are O(8))
  O^T   = V.T @ P^T accumulated over s in PSUM (row 64 = denominator)
  y     = (O^T / denom)^T @ wo-slice^T, partial over this core's heads

The attention stage is ScalarE(exp)-bound (~16.8M exp/core), so the s-loop
is software-pipelined in chunks of 2 s-tiles: the PE issues chunk c+1's
score matmuls before chunk c's PV matmuls so the in-order PE queue never
stalls on the exp dependency, and each exp is one [128,1024] PSUM-source
ACTIVATE (amortizes the per-instruction bubble). Final projection and the
softmax normalization for query-block tb are deferred into tb+1's pipeline.
"""

import sys

sys.path.insert(0, "/opt/trn_rl_repo")

from contextlib import ExitStack

import ml_dtypes
import numpy as np

import concourse.bass as bass
import concourse.tile as tile
from concourse import bacc, mybir
from concourse import bass_utils
from concourse.bass_interp import get_hw_module

BF16 = mybir.dt.bfloat16
F32 = mybir.dt.float32
EXP = mybir.ActivationFunctionType.Exp

N_EMBD = 1024
N_HEAD = 16
HEAD_DIM = 64

N_CORES = 8
HEADS_PER_CORE = 4
DH = HEADS_PER_CORE * HEAD_DIM  # 256
CH = 2  # s-tiles per exp chunk


def build_program(T=2048, C=N_EMBD, enable_asserts=False):
    nc = bacc.Bacc(
        "TRN2", target_bir_lowering=False, debug=False, enable_asserts=enable_asserts
    )

    xT = nc.dram_tensor("xT", [C, T], BF16, kind="ExternalInput").ap()
    wqT = nc.dram_tensor("wqT", [C, DH], BF16, kind="ExternalInput").ap()
    wkT = nc.dram_tensor("wkT", [C, DH], BF16, kind="ExternalInput").ap()
    wvT = nc.dram_tensor("wvT", [C, DH], BF16, kind="ExternalInput").ap()
    woT = nc.dram_tensor("woT", [DH, C], BF16, kind="ExternalInput").ap()
    y = nc.dram_tensor("y", [T, C], F32, kind="ExternalOutput").ap()

    n_ct = C // 128   # contraction tiles over embedding dim
    n_st = T // 128   # s tiles (key/value positions)
    n_tb = T // 512   # query blocks
    n_cb = C // 512   # output column blocks
    n_ck = n_st // CH  # exp chunks per (h, tb)

    scale = float(HEAD_DIM**-0.5)

    with tile.TileContext(nc) as tc, ExitStack() as ctx:
        statics = ctx.enter_context(tc.tile_pool(name="statics", bufs=1))
        pt_pool = ctx.enter_context(tc.tile_pool(name="pt", bufs=4))
        onorm_pool = ctx.enter_context(tc.tile_pool(name="onorm", bufs=6))
        small = ctx.enter_context(tc.tile_pool(name="small", bufs=6))
        out_stage = ctx.enter_context(tc.tile_pool(name="out_stage", bufs=4))

        psum_s = ctx.enter_context(tc.tile_pool(name="psum_s", bufs=2, space="PSUM"))
        psum_o = ctx.enter_context(tc.tile_pool(name="psum_o", bufs=2, space="PSUM"))
        psum_f = ctx.enter_context(tc.tile_pool(name="psum_f", bufs=2, space="PSUM"))

        # ---- static SBUF tensors ----
        xT_sb = statics.tile([128, n_ct, T], BF16)
        wq_sb = statics.tile([128, n_ct, DH], BF16)
        wk_sb = statics.tile([128, n_ct, DH], BF16)
        wv_sb = statics.tile([128, n_ct, DH], BF16)
        wo_sb = statics.tile([128, 2, C], BF16)
        qT_sb = statics.tile([128, 2, T], BF16)
        kT_sb = statics.tile([128, 2, T], BF16)
        v_sb = statics.tile([128, n_st, HEADS_PER_CORE, HEAD_DIM + 1], BF16)
        oT_sb = statics.tile([128, 2, T], BF16)
        ones_sb = statics.tile([1, 64], BF16)

        nc.sync.dma_start(out=wk_sb, in_=wkT.rearrange("(a p) d -> p a d", p=128))
        nc.sync.dma_start(out=wv_sb, in_=wvT.rearrange("(a p) d -> p a d", p=128))
        nc.sync.dma_start(out=wq_sb, in_=wqT.rearrange("(a p) d -> p a d", p=128))
        nc.sync.dma_start(out=wo_sb, in_=woT.rearrange("(a p) c -> p a c", p=128))
        xT_chunked = xT.rearrange("(a p) t -> p a t", p=128)
        for ct in range(n_ct):
            nc.sync.dma_start(out=xT_sb[:, ct, :], in_=xT_chunked[:, ct, :])
        nc.vector.memset(ones_sb, 1.0)
        nc.vector.memset(v_sb[:, :, :, HEAD_DIM : HEAD_DIM + 1], 1.0)

        # ---- projections (K, V first; attention waits on all of them) ----
        qk_projs = ((wk_sb, kT_sb), (wq_sb, qT_sb))
        for w_sb, dst in qk_projs[:1]:
            for chunk in range(2):
                for tb in range(n_tb):
                    ps = psum_f.tile([128, 512], F32, tag="f")
                    for ct in range(n_ct):
                        nc.tensor.matmul(
                            ps,
                            lhsT=w_sb[:, ct, chunk * 128 : (chunk + 1) * 128],
                            rhs=xT_sb[:, ct, tb * 512 : (tb + 1) * 512],
                            start=(ct == 0),
                            stop=(ct == n_ct - 1),
                        )
                    nc.vector.tensor_copy(dst[:, chunk, tb * 512 : (tb + 1) * 512], ps)

        for st in range(n_st):
            ps = psum_f.tile([128, 512], F32, tag="f")
            for ct in range(n_ct):
                nc.tensor.matmul(
                    ps[:, 0:DH],
                    lhsT=xT_sb[:, ct, st * 128 : (st + 1) * 128],
                    rhs=wv_sb[:, ct, :],
                    start=(ct == 0),
                    stop=(ct == n_ct - 1),
                )
            nc.vector.tensor_copy(
                v_sb[:, st, :, 0:HEAD_DIM],
                ps[:, 0:DH].rearrange("p (h d) -> p h d", h=HEADS_PER_CORE),
            )

        for w_sb, dst in qk_projs[1:]:
            for chunk in range(2):
                for tb in range(n_tb):
                    ps = psum_f.tile([128, 512], F32, tag="f")
                    for ct in range(n_ct):
                        nc.tensor.matmul(
                            ps,
                            lhsT=w_sb[:, ct, chunk * 128 : (chunk + 1) * 128],
                            rhs=xT_sb[:, ct, tb * 512 : (tb + 1) * 512],
                            start=(ct == 0),
                            stop=(ct == n_ct - 1),
                        )
                    nc.vector.tensor_copy(dst[:, chunk, tb * 512 : (tb + 1) * 512], ps)

        # ---- attention: software-pipelined chunk loop ----
        # chunk records: (h, tb, c, ps, pt, o_ps); deferred: list of
        # (slot_index, emit_fn) executed at the start of pipeline slot i.
        heads = list(range(HEADS_PER_CORE))
        chunk_list = []
        for tb in range(n_tb):
            for h in heads:
                for c in range(n_ck):
                    chunk_list.append((tb, h, c))
        n_total = len(chunk_list)

        deferred = {}

        def defer(slot, fn):
            deferred.setdefault(slot, []).append(fn)

        o_ps_map = {}
        denom_map = {}

        def emit_S_ACT(i):
            tb, h, c = chunk_list[i]
            chunk_hd, dlo = h // 2, (h % 2) * 64
            if c == 0:
                o_ps = psum_o.tile([HEAD_DIM + 1, 512], F32, tag="o")
                o_ps_map[(tb, h)] = o_ps
            ps = psum_s.tile([128, CH * 512], F32, tag="s")
            for j in range(CH):
                st = c * CH + j
                nc.tensor.matmul(
                    ps[:, j * 512 : (j + 1) * 512],
                    lhsT=kT_sb[dlo : dlo + 64, chunk_hd, st * 128 : (st + 1) * 128],
                    rhs=qT_sb[dlo : dlo + 64, chunk_hd, tb * 512 : (tb + 1) * 512],
                    start=True,
                    stop=True,
                )
            pt = pt_pool.tile([128, CH, 512], BF16)
            nc.scalar.activation(
                pt.rearrange("p a b -> p (a b)"), ps, EXP, scale=scale
            )
            return ps, pt

        def emit_O(i, pt):
            tb, h, c = chunk_list[i]
            o_ps = o_ps_map[(tb, h)]
            for j in range(CH):
                st = c * CH + j
                nc.tensor.matmul(
                    o_ps,
                    lhsT=v_sb[:, st, h, :],
                    rhs=pt[:, j, :],
                    start=(c == 0 and j == 0),
                    stop=(c == n_ck - 1 and j == CH - 1),
                )
            if c == n_ck - 1:
                # head (tb, h) fully accumulated: stash unnormalized O
                o_unnorm = onorm_pool.tile([64, 512], BF16, tag="ou")
                nc.vector.tensor_copy(o_unnorm, o_ps[0:64, :])
                o_unnorm_map[(tb, h)] = o_unnorm

        o_unnorm_map = {}

        def emit_norm(tb, h):
            chunk_hd, dlo = h // 2, (h % 2) * 64
            o_ps = o_ps_map[(tb, h)]
            denom_f = small.tile([1, 512], F32, tag="denom_f")
            nc.vector.tensor_copy(denom_f, o_ps[64:65, :])
            recip_f = small.tile([1, 512], F32, tag="recip_f")
            nc.vector.reciprocal_approx_fast(recip_f, denom_f)
            recip = small.tile([1, 512], BF16, tag="recip")
            nc.vector.tensor_copy(recip, recip_f)
            rep = psum_f.tile([128, 512], F32, tag="f")
            nc.tensor.matmul(
                rep[0:64, :], lhsT=ones_sb, rhs=recip, start=True, stop=True
            )
            rep_sb = small.tile([64, 512], BF16, tag="rep")
            nc.vector.tensor_copy(rep_sb, rep[0:64, :])
            nc.vector.tensor_mul(
                oT_sb[dlo : dlo + 64, chunk_hd, tb * 512 : (tb + 1) * 512],
                o_unnorm_map[(tb, h)],
                rep_sb,
            )

        def emit_final(tb):
            # y partial for query block tb: 4 t-tiles of 128
            y_tiled = y.rearrange("(tt p) c -> p tt c", p=128)
            for tt in range(tb * 4, tb * 4 + 4):
                for cb in range(n_cb):
                    ps = psum_f.tile([128, 512], F32, tag="f")
                    for chunk in range(2):
                        nc.tensor.matmul(
                            ps,
                            lhsT=oT_sb[:, chunk, tt * 128 : (tt + 1) * 128],
                            rhs=wo_sb[:, chunk, cb * 512 : (cb + 1) * 512],
                            start=(chunk == 0),
                            stop=(chunk == 1),
                        )
                    st_out = out_stage.tile([128, 512], F32)
                    nc.vector.tensor_copy(st_out, ps)
                    nc.sync.dma_start(
                        out=y_tiled[:, tt, cb * 512 : (cb + 1) * 512], in_=st_out
                    )

        # schedule: at slot i emit S+ACT(i) then O(i-1); normalization for tb
        # lands ~3 chunks into tb+1, final projection ~6 chunks in.
        per_tb = HEADS_PER_CORE * n_ck
        for tb in range(n_tb):
            for h in heads:
                head_end = (tb * HEADS_PER_CORE + h + 1) * n_ck
                last = n_total + 1  # after the final pending O pops
                defer(min(head_end + 3, last), lambda tb=tb, h=h: emit_norm(tb, h))
            end_slot = (tb + 1) * per_tb
            defer(min(end_slot + 6, last), lambda tb=tb: emit_final(tb))

        from collections import deque

        pending = deque()
        DEPTH = 2
        for i in range(n_total + DEPTH):
            if i < n_total:
                ps, pt = emit_S_ACT(i)
                pending.append((i, pt))
            if len(pending) > DEPTH or i >= n_total:
                j, jpt = pending.popleft()
                emit_O(j, jpt)
            for fn in deferred.get(i, ()):
                fn()

    nc.compile()
    return nc


def make_core_inputs(x, wq, wk, wv, wo):
    """Shard + pre-layout the full inputs into 8 per-core input maps."""
    bf = ml_dtypes.bfloat16
    in_maps = []
    for core in range(N_CORES):
        b = core // 4
        g = core % 4
        lo, hi = g * DH, (g + 1) * DH
        in_maps.append(
            {
                "xT": np.ascontiguousarray(x[b].T).astype(bf),
                "wqT": np.ascontiguousarray(wq[lo:hi, :].T).astype(bf),
                "wkT": np.ascontiguousarray(wk[lo:hi, :].T).astype(bf),
                "wvT": np.ascontiguousarray(wv[lo:hi, :].T).astype(bf),
                "woT": np.ascontiguousarray(wo[:, lo:hi].T).astype(bf),
            }
        )
    return in_maps


_PROGRAM_CACHE = {}


def _get_program():
    if "nc" not in _PROGRAM_CACHE:
        nc = build_program()
        nc.m = get_hw_module(nc.m)
        _PROGRAM_CACHE["nc"] = nc
    return _PROGRAM_CACHE["nc"]


def run_sharded(in_maps, trace=False):
    nc = _get_program()
    return bass_utils.run_bass_kernel_spmd(
        nc, in_maps, core_ids=list(range(N_CORES)), trace=trace
    )


def kernel(x, wq, wk, wv, wo):
    x = np.asarray(x, dtype=np.float32)
    wq = np.asarray(wq, dtype=np.float32)
    wk = np.asarray(wk, dtype=np.float32)
    wv = np.asarray(wv, dtype=np.float32)
    wo = np.asarray(wo, dtype=np.float32)

    in_maps = make_core_inputs(x, wq, wk, wv, wo)
    res = run_sharded(in_maps)

    B, T, C = x.shape
    out = np.zeros((B, T, C), dtype=np.float32)
    for core in range(N_CORES):
        out[core // 4] += res.results[core]["y"]
    return out


if __name__ == "__main__":
    rng = np.random.default_rng(0)
    x = rng.standard_normal((2, 2048, 1024), dtype=np.float32)
    s = 1.0 / np.sqrt(N_EMBD)
    ws = [rng.standard_normal((1024, 1024), dtype=np.float32) * s for _ in range(4)]
    out = kernel(x, *ws)
    print("out", out.shape, out.dtype, float(np.abs(out).max()))
